# revision 43
# baseline (speedup 1.0000x reference)
"""Trainium2 Bass kernel for nn_Attention_56169582297517.

ref:  q = primary @ W.T + b            [N,L]
      k = secondary @ W.T + b          [M,L]
      s = relu(q @ k.T)                [N,M]
      s = s / max(||s||_row, 1e-12)
      out = s @ secondary              [N,E]

N=M=8192, E=512, L=128.  Sharding: primary rows split across 8 cores
(1024 rows each); secondary/W/b replicated; each core computes its row
slice independently (row-wise L2 norm is local to N).

Per-core plan (normalization deferred to the very end):
  out_row = (relu(q k^T) @ S)_row / max(norm_row, eps)

Scores are computed TRANSPOSED (m on partitions, n on free) so the
context matmul can contract m on partitions against natural-layout
secondary chunks.

The context matmul runs in fp8e4 with perf_mode=DoubleRow: two m-chunks
packed per matmul ([K=128, 2, *] APs on both operands), doubling
contraction throughput.  Scores are relu'd + cast to fp8 by the DVE
(ACT cannot write fp8 - hangs the exec unit); max score ~112 < 240 so
no scaling is needed.

The row norm is ALSO computed on the PE: a second DoubleRow matmul per
(pair, n-block) accumulates the gram diag blocks st8^T @ st8 into one
PSUM bank; the diagonal (= sum_m s^2 over the same fp8 values the ctx
matmul uses) is extracted at group end by scalar_tensor_tensor against
the identity with accum_out.

vs the 234us baseline (now ~202us):
- All inputs stream in as bf16 via SWDGE cast-DMAs (gpsimd ring): the
  f32->bf16 casts ride the DMA CME units, removing the per-superchunk
  ACT bf16-cast (1.9us) and halving the SBUF-side DMA write bytes.
- s8 (the fp8 secondary for the ctx matmul) loads DIRECTLY as a second
  SWDGE f32->fp8 cast-DMA.  A DVE cast op would sit in the strict-FIFO
  DVE queue ahead of later relus; the Tile scheduler (whose cost model
  underestimates SWDGE completion latency) hoists those far ahead, and
  on HW each late load then head-of-line blocks the relu stream and
  stalls the PE 2-4us (and the idles re-throttle the PE clock 2.4 ->
  1.2GHz, compounding).  The extra 16MB HBM read rides free bandwidth.
- The ENTIRE bf16 secondary is prefetched into SBUF (64KB/partition,
  no s_bf buffer reuse): load descriptor-gen is never gated on PE
  progress (the reuse dependency chained Q7 gen to the PE semaphore and
  collapsed the pipeline to a 10us/superchunk lockstep).
- All transposes on the PE.  XBAR dma-transpose is unusable here: each
  InstDmaTranspose acts as a barrier in the scheduled DMA stream (waits
  for all prior DMAs, blocks all later ones), serializing against the
  continuous load traffic, and concurrent XBAR transposes on both HWDGE
  rings race and corrupt data.
- gram matmuls are NOT DoubleRow: a DR LDWEIGHTS loads 256 columns
  (~213ns) against a ~53ns FD=128 matmul; with the 4 ctx DR loads per
  pair already in flight this made pairs LDWEIGHTS-pipe-bound (2.06us
  vs 1.82 matmul-stream).  Two plain fp8 accumulating matmuls per
  block get FWL (~27ns) instead.
- The proj PSUM bank rotates kproj(sc+1) -> T(sc+2,jp1) -> T(sc+3,jp0)
  across one loop iteration so every tenant's ACT drain hides behind a
  full pair of matmuls (the v1 phase-lock stalled the PE 780ns/SC).
- b loads as one contiguous [1,128] descriptor and is transposed to
  [128,1] by a K=1 PE matmul (was: 128 4-byte DMA descriptors).
- Output stores batched: one [128, NB, E] tile per group -> 8KB
  contiguous per partition, split in two halves so the first store
  overlaps the second half's scaling.
- qproj h1 runs inside the last group-0 iteration (group 1 is its only
  consumer), off the startup critical path.
- Group boundary: the next group's prologue (scores) is emitted before
  the previous group's finalize, and the last pair's gram matmuls are
  emitted before its ctx matmuls, so the norm/scale/store chain
  overlaps the next group's score/relu pipeline.

PSUM budget (8 banks): proj(1) + scores(2) + ctx(4) + gram(1) = 8.
"""

import sys
import types

import numpy as np
from contextlib import ExitStack

import concourse.bass as bass
import concourse.bacc as bacc
import concourse.mybir as mybir
import concourse.tile as tile
from concourse.bass_utils import run_bass_kernel_spmd
from concourse.masks import make_identity


def _install_ntff_shim():
    """Some images lack antenv.axon_hooks; synthesize it so
    run_bass_kernel_spmd(trace=True) (or BASS_TRACE=1) can't crash on the
    import, and wire the NTFF profile hook when the axon .so supports it."""
    if "antenv.axon_hooks" in sys.modules:
        return
    try:
        import antenv
        import antenv.axon_hooks  # noqa: F401
        return  # real module exists
    except ImportError:
        pass
    try:
        mod = types.ModuleType("antenv.axon_hooks")
        mod._hook = None
        mod.set_axon_ntff_profile_hook = lambda h: setattr(mod, "_hook", h)
        mod.get_axon_ntff_profile_hook = lambda: mod._hook
        sys.modules["antenv.axon_hooks"] = mod
        antenv.axon_hooks = mod
        try:
            from trn_agent_boot.trn_boot import _ntff_profile_via_ctypes

            hook = _ntff_profile_via_ctypes("/opt/axon/libaxon_pjrt.so")
            if hook is not None:
                mod.set_axon_ntff_profile_hook(hook)
        except Exception:
            pass
    except Exception:
        pass


_install_ntff_shim()

N_CORES = 8
N, M, E, L = 8192, 8192, 512, 128
NLOC = N // N_CORES          # 1024 primary rows per core
P = 128
EC = E // P                  # 4 e-chunks of 128
M_CHUNKS = M // P            # 64 m-chunks of 128
M_PAIRS = M_CHUNKS // 2      # 32 fp8 DoubleRow pairs
SC = 4                       # m-chunks per load superchunk (512 rows)
N_SUPER = M_CHUNKS // SC     # 16
PPS = SC // 2                # pairs per superchunk (2)
NG = 512                     # n-group width (psum free dim)
N_GROUPS = NLOC // NG        # 2
NB = NG // P                 # 4 n-blocks of 128 per group
EPS = 1e-12

F32 = mybir.dt.float32
BF16 = mybir.dt.bfloat16
FP8 = mybir.dt.float8e4
AF = mybir.ActivationFunctionType
ALU = mybir.AluOpType
DR = mybir.MatmulPerfMode.DoubleRow


def _emit(nc: bass.Bass):
    prim = nc.dram_tensor("primary", [NLOC, E], F32, kind="ExternalInput")
    sec = nc.dram_tensor("secondary", [M, E], F32, kind="ExternalInput")
    w_d = nc.dram_tensor("W", [L, E], F32, kind="ExternalInput")
    b_d = nc.dram_tensor("b", [L], F32, kind="ExternalInput")
    out_d = nc.dram_tensor("out", [NLOC, E], F32, kind="ExternalOutput")

    with tile.TileContext(nc) as tc, ExitStack() as ctx:
        consts = ctx.enter_context(tc.tile_pool(name="consts", bufs=1))
        big = ctx.enter_context(tc.tile_pool(name="big", bufs=1))
        stage = ctx.enter_context(tc.tile_pool(name="stage", bufs=2))
        work = ctx.enter_context(tc.tile_pool(name="work", bufs=3))
        psum = ctx.enter_context(tc.tile_pool(name="psum", bufs=1, space="PSUM"))

        # ---------------- constants ----------------
        ident = consts.tile([P, P], F32)
        make_identity(nc, ident)
        ident_bf = consts.tile([P, P], BF16)
        make_identity(nc, ident_bf)

        # W: one SWDGE cast-load (f32->bf16) + PE transposes.  NO XBAR
        # dma-transposes anywhere: each one acts as a barrier in the
        # scheduled DMA stream (it waits for all previously scheduled DMAs
        # and blocks all later ones), which serializes against the
        # continuous SWDGE load traffic.
        w_bf = consts.tile([P, E], BF16)
        nc.gpsimd.dma_start(w_bf, w_d[:])
        wt = consts.tile([P, EC, P], BF16)       # wt[e', ec, l] = W[l, ec*128+e']
        for e in range(EC):
            tp = psum.tile([P, P], BF16, tag="gram", name="tp")
            nc.tensor.transpose(tp, w_bf[:, e * P:(e + 1) * P], ident_bf)
            nc.scalar.copy(wt[:, e, :], tp)

        # b: contiguous [1,128] load, then a K=1 matmul against ident[0,0]=1
        # puts b on partitions ([128,1]) for the activation bias reads
        b_row = consts.tile([1, L], F32)
        nc.sync.dma_start(b_row, b_d[:].rearrange("(o l) -> o l", o=1))
        b_ps = psum.tile([P, 1], F32, tag="gram", name="b_ps")
        nc.tensor.matmul(b_ps, lhsT=b_row, rhs=ident[0:1, 0:1], start=True, stop=True)
        b_sb = consts.tile([P, 1], F32)
        nc.scalar.copy(b_sb, b_ps)

        # ------------- secondary stream state -------------
        s8 = big.tile([P, M_PAIRS, 2, E], FP8)     # [m_in, pair, j, e]
        kt = big.tile([P, M], BF16)                # [l, m]
        s_bfs = {}
        st_sbs = {}

        def emit_load(sc):
            # SWDGE cast-DMA: f32 HBM -> bf16 SBUF.  Partition p holds DRAM
            # rows 4p+j (j inner): 8KB contiguous reads per partition.  One
            # 1MB dma per superchunk: SWDGE descriptor-gen is paced by lane
            # reuse (8 lanes x ~2us completion), so fewer/bigger ops drain
            # the upfront prefetch queue faster.  The m<->partition
            # permutation is absorbed by construction: kt columns, st8
            # partitions and s8 partitions all inherit it from this same
            # load, and m is fully contracted.
            s_bf = stage.tile([P, SC, E], BF16, tag="sbf", name="s_bf", bufs=16)
            base = sec[sc * SC * P:(sc + 1) * SC * P, :].rearrange("(p j) e -> p j e", j=SC)
            nc.gpsimd.dma_start(s_bf, base)
            s_bfs[sc] = s_bf
            emit_s8load(sc)

        def emit_T(sc, jp, t_tag="proj"):
            # PE transposes of two m-chunks.  The proj PSUM bank rotates
            # kproj(sc+1) -> T(sc+2,jp1) -> T(sc+3,jp0) across one loop
            # iteration, so every tenant's drain has a full pair of score/
            # ctx matmuls in front of the next tenant's first write - the
            # v1 phase-lock (T, drain, T, drain, kproj back-to-back) stalled
            # the PE 780ns+ per superchunk waiting on the ACT drains.
            s_bf = s_bfs[sc]
            st_ps = psum.tile([P, EC, 2 * P], BF16, tag=t_tag, name="st_ps")
            for jj in range(2):
                j = jp * 2 + jj
                for e in range(EC):
                    nc.tensor.transpose(
                        st_ps[:, e, jj * P:(jj + 1) * P],
                        s_bf[:, j, e * P:(e + 1) * P],
                        ident_bf,
                    )
            if sc not in st_sbs:
                st_sbs[sc] = stage.tile([P, EC, SC * P], BF16, tag="st", name="st_sb", bufs=3)
            dst = st_sbs[sc][:, :, jp * 2 * P:(jp + 1) * 2 * P]
            # both drains on ACT: the DVE queue must stay pure relu+s8cast,
            # a drain queued behind the relus would stall kproj on the PE
            nc.scalar.copy(dst, st_ps)

        def emit_kproj(sc):
            st_sb = st_sbs.pop(sc)
            pk = psum.tile([P, SC * P], F32, tag="proj", name="pk")
            for e in range(EC):
                nc.tensor.matmul(
                    pk,
                    lhsT=wt[:, e, :],
                    rhs=st_sb[:, e, :],
                    start=(e == 0),
                    stop=(e == EC - 1),
                )
            nc.scalar.activation(kt[:, sc * SC * P:(sc + 1) * SC * P], pk, AF.Identity, bias=b_sb)

        def emit_s8load(sc):
            # s8 loads DIRECTLY as a second SWDGE cast-DMA (f32 HBM -> fp8
            # SBUF).  A DVE bf16->fp8 cast op would sit in the strict-FIFO
            # DVE queue ahead of later relus; the Tile scheduler (whose cost
            # model underestimates SWDGE completion latency) hoists these
            # far ahead, and on HW each late load then head-of-line blocks
            # the relu stream and stalls the PE 2-4us.  The extra 16MB HBM
            # read rides free bandwidth.
            base = sec[sc * SC * P:(sc + 1) * SC * P, :].rearrange("(p j) e -> p j e", j=SC)
            nc.gpsimd.dma_start(s8[:, sc * PPS:(sc + 1) * PPS, :, :], base)

        # ---------------- qT = W @ P_loc^T + b  -> [l, n]  (bf16) ----------------
        # SWDGE cast-load (partition p holds prim rows h*512+4p+j -> qt
        # column j*128+p), PE transposes (gram bank is idle until the main
        # loop), then the e-contraction on PE.
        qt = big.tile([P, NLOC], BF16)
        pc_bfs = []
        for h in range(NLOC // NG):
            # Pool-ring order pc0, L0, pc1, L1: each consumer's load lands
            # just ahead of its first use in the startup chain
            pc_bf = stage.tile([P, NB, E], BF16, tag="pchunk", name="pc_bf")
            nc.gpsimd.dma_start(
                pc_bf, prim[h * NG:(h + 1) * NG, :].rearrange("(p j) e -> p j e", j=NB))
            pc_bfs.append(pc_bf)
            emit_load(h)

        def emit_qproj(h, ps_tags=("gram", "proj"), interleave=False):
            # alternate PSUM banks for the transpose blocks so consecutive
            # drains overlap (a single bank serializes T -> drain -> T ...).
            # interleave=True (startup h0 only): pq lives in a scores bank
            # (free at startup) and each block's projection matmuls are
            # emitted right after the previous block's drain, filling every
            # bank-handoff window with PE work.  Not for h1: the scores
            # banks are hot mid-loop, and pq-at-top in the proj rotation
            # would deadlock against the later pt blocks.
            pc_bf = pc_bfs[h]
            pt_sb = stage.tile([P, NB, EC, P], BF16, tag="pt", name="pt_sb")
            pq = psum.tile([P, NG], F32, tag="scores" if interleave else "proj",
                           name="pq", bufs=2 if interleave else 1)

            def pq_block(nb4):
                for e in range(EC):
                    nc.tensor.matmul(
                        pq[:, nb4 * P:(nb4 + 1) * P],
                        lhsT=wt[:, e, :],
                        rhs=pt_sb[:, nb4, e, :],
                        start=(e == 0),
                        stop=(e == EC - 1),
                    )

            for nb4 in range(NB):
                pt_ps = psum.tile([P, EC, P], BF16, tag=ps_tags[nb4 % len(ps_tags)], name="pt_ps")
                for e in range(EC):
                    nc.tensor.transpose(
                        pt_ps[:, e, :], pc_bf[:, nb4, e * P:(e + 1) * P], ident_bf)
                if nb4 % 2 == 0:
                    nc.scalar.copy(pt_sb[:, nb4, :, :], pt_ps)
                else:
                    nc.vector.tensor_copy(pt_sb[:, nb4, :, :], pt_ps)
                if interleave and nb4 >= 1:
                    pq_block(nb4 - 1)
            if interleave:
                pq_block(NB - 1)
            else:
                for nb4 in range(NB):
                    pq_block(nb4)
            nc.scalar.activation(qt[:, h * NG:(h + 1) * NG], pq, AF.Identity, bias=b_sb)

        # ---------------- main loop: scores^T, gram norms, context ----------------
        g1_sc_count = {"n": 0}

        def emit_scores_pair(g, mp):
            # group 1 has no kproj/transposes, so the proj bank is free:
            # rotate scores through THREE banks (scores x2 + proj) - each
            # score matmul then waits the relu from three tiles back
            # instead of two, relieving the scores<->relu ping-pong that
            # exposes ~0.5us/superchunk of semaphore latency
            tiles = []
            for j in range(2):
                if g == 1:
                    k = g1_sc_count["n"]
                    g1_sc_count["n"] += 1
                    tag = "scores" if k % 3 < 2 else "proj"
                else:
                    tag = "scores"
                sc_ps = psum.tile([P, NG], F32, tag=tag, name="sc_ps",
                                  bufs=2 if tag == "scores" else 1)
                nc.tensor.matmul(
                    sc_ps,
                    lhsT=kt[:, (2 * mp + j) * P:(2 * mp + j + 1) * P],
                    rhs=qt[:, g * NG:(g + 1) * NG],
                    start=True,
                    stop=True,
                )
                tiles.append(sc_ps)
            return tiles

        def emit_group_prologue(g):
            ctx_ps = [
                psum.tile([P, E], F32, tag=f"ctx{jb}", name=f"ctx{jb}") for jb in range(NB)
            ]
            gram_ps = psum.tile([P, NB * P], F32, tag="gram", name="gram_ps")
            return {"ctx_ps": ctx_ps, "gram_ps": gram_ps,
                    "sc": emit_scores_pair(g, 0)}

        def emit_pair(g, st, mp, split_relu=False):
            st8 = work.tile([P, 2, NG], FP8, tag="st8", name="st8", bufs=4)
            # relu + fp8 cast on DVE (ACT cannot write fp8).  In group 0 the
            # relu is split into column halves: the first ctx/gram matmuls
            # read only st8[:, :, 0:256], so the PE's per-superchunk relu
            # edge shrinks by ~half a relu.  Group 1 keeps whole-row relus
            # (its pair cadence is tighter and the extra DVE op overhead
            # would make the DVE the bottleneck).
            H = NG // 2
            if split_relu:
                nc.vector.tensor_scalar_max(st8[:, 0, 0:H], st["sc"][0][:, 0:H], 0.0)
                nc.vector.tensor_scalar_max(st8[:, 1, 0:H], st["sc"][1][:, 0:H], 0.0)
                nc.vector.tensor_scalar_max(st8[:, 0, H:NG], st["sc"][0][:, H:NG], 0.0)
                nc.vector.tensor_scalar_max(st8[:, 1, H:NG], st["sc"][1][:, H:NG], 0.0)
            else:
                nc.vector.tensor_scalar_max(st8[:, 0, :], st["sc"][0], 0.0)
                nc.vector.tensor_scalar_max(st8[:, 1, :], st["sc"][1], 0.0)
            # next pair's scores issued ahead of the ctx matmuls in the
            # in-order PE stream (they run as soon as the relus drain the
            # score banks, feeding the next relus)
            if mp + 1 < M_PAIRS:
                st["sc"] = emit_scores_pair(g, mp + 1)
            last = mp == M_PAIRS - 1

            def emit_ctx(jb, lhsT):
                nc.tensor.matmul(
                    st["ctx_ps"][jb],
                    lhsT=lhsT,
                    rhs=s8[:, mp, :, :],
                    start=(mp == 0),
                    stop=last,
                    perf_mode=DR,
                )

            def emit_gram(jb, lhsT):
                # row-norm accumulation: gram diag block, same values as the
                # ctx lhsT.  Mixed perf modes: a DR LDWEIGHTS loads 256
                # columns (~213ns) vs a ~53ns matmul at FD=128, so all-DR
                # made pairs LDWEIGHTS-pipe-bound (2.06us vs 1.82 matmul
                # stream) while all-plain doubles the gram matmul cycles.
                # Two DR blocks + two plain-FWL blocks lands both streams
                # under the pipe limits (LDW 1.49us, MM 1.66us per pair).
                # The PSUM zero region is the whole 2KB bank, so only the
                # FIRST gram matmul may carry start=True: a start on a later
                # one would clear has_written for the already-written
                # regions and their next write would overwrite, silently
                # dropping pair 0 from those rows' norms.
                if jb % 2 == 0:
                    nc.tensor.matmul(
                        st["gram_ps"][:, jb * P:(jb + 1) * P],
                        lhsT=lhsT,
                        rhs=lhsT,
                        start=(mp == 0 and jb == 0),
                        stop=False,
                        perf_mode=DR,
                        skip_group_check=True,
                    )
                else:
                    for j in range(2):
                        nc.tensor.matmul(
                            st["gram_ps"][:, jb * P:(jb + 1) * P],
                            lhsT=lhsT[:, j, :],
                            rhs=lhsT[:, j, :],
                            start=False,
                            stop=(last and jb == NB - 1 and j == 1),
                            skip_group_check=True,
                        )

            for jb in range(NB):
                lhsT = st8[:, :, jb * P:(jb + 1) * P]
                if last:
                    # gram first on the last pair: the gram bank completes
                    # ~0.9us earlier, so the finalize norm chain overlaps
                    # the remaining ctx matmuls
                    emit_gram(jb, lhsT)
                else:
                    emit_ctx(jb, lhsT)
                    emit_gram(jb, lhsT)
            if last:
                for jb in range(NB):
                    emit_ctx(jb, st8[:, :, jb * P:(jb + 1) * P])

        def emit_group_finalize(g, st, split_copies=False):
            # ------- out = ctx / max(sqrt(diag(gram)), eps) -------
            o_raw = None
            if not split_copies:
                # raw ctx drains first: free the banks for the next group.
                # Split ACT/DVE - these are gated only on the ctx matmuls
                # (not the norm chain), so the DVE ops cannot head-of-line
                # block the next group's relus
                o_raw = work.tile([P, NB, E], F32, tag="oraw", name="o_raw", bufs=1)
                nc.scalar.copy(o_raw[:, 0, :], st["ctx_ps"][0])
                nc.vector.tensor_copy(o_raw[:, 2, :], st["ctx_ps"][2])
                nc.scalar.copy(o_raw[:, 1, :], st["ctx_ps"][1])
                nc.vector.tensor_copy(o_raw[:, 3, :], st["ctx_ps"][3])
            n2 = work.tile([P, NB], F32, tag="n2", name="n2", bufs=1)
            for jb in range(NB):
                scratch = work.tile([P, P], F32, tag="scr", name="scratch", bufs=2)
                nc.vector.scalar_tensor_tensor(
                    scratch, st["gram_ps"][:, jb * P:(jb + 1) * P], 1.0, ident,
                    ALU.mult, ALU.mult, accum_out=n2[:, jb:jb + 1],
                )
            nrm = work.tile([P, NB], F32, tag="nrm", name="nrm", bufs=1)
            nc.scalar.activation(nrm, n2, AF.Sqrt)
            nrm_c = work.tile([P, NB], F32, tag="nrmc", name="nrm_c", bufs=1)
            nc.vector.tensor_scalar_max(nrm_c, nrm, EPS)
            recip = work.tile([P, NB], F32, tag="recip", name="recip", bufs=1)
            nc.vector.reciprocal(recip, nrm_c)
            # batched store: o_sb partition p holds rows g*512 + 4p + jb (the
            # primary-load permutation) -> 8KB contiguous per partition,
            # split in two halves so store 0 overlaps the jb=2,3 scaling
            o_sb = work.tile([P, NB, E], F32, tag="osb", name="o_sb", bufs=2)
            out_blk = out_d[g * NG:(g + 1) * NG, :].rearrange("(p j) e -> p j e", j=NB)
            if split_copies:
                # final group only: scale-copies split ACT (jb 0,1) / DVE
                # (jb 2,3) so the two halves run in parallel - the serial
                # 2.9us ACT chain was the fattest piece of the exit tail.
                # NOT for the mid-kernel finalize: a DVE op gated on the
                # norm chain would head-of-line block the next group's
                # relus in the strict-FIFO DVE queue.
                nc.scalar.activation(o_sb[:, 0, :], st["ctx_ps"][0], AF.Copy,
                                     scale=recip[:, 0:1])
                nc.vector.tensor_scalar_mul(o_sb[:, 2, :], st["ctx_ps"][2], recip[:, 2:3])
                nc.scalar.activation(o_sb[:, 1, :], st["ctx_ps"][1], AF.Copy,
                                     scale=recip[:, 1:2])
                nc.vector.tensor_scalar_mul(o_sb[:, 3, :], st["ctx_ps"][3], recip[:, 3:4])
                nc.scalar.dma_start(out_blk[:, 0:2, :], o_sb[:, 0:2, :])
                nc.scalar.dma_start(out_blk[:, 2:4, :], o_sb[:, 2:4, :])
            else:
                # mid-kernel finalize: drain the ctx banks RAW first (gated
                # only on the last ctx matmul, not on the norm chain) so the
                # next group's start=True ctx matmuls get their banks ~3us
                # earlier; the scale then applies SBUF->SBUF off-path
                for jb in range(NB):
                    nc.scalar.activation(o_sb[:, jb, :], o_raw[:, jb, :], AF.Copy,
                                         scale=recip[:, jb:jb + 1])
                    if jb == 1:
                        nc.scalar.dma_start(out_blk[:, 0:2, :], o_sb[:, 0:2, :])
                nc.scalar.dma_start(out_blk[:, 2:4, :], o_sb[:, 2:4, :])

        # Phase-0 production interleaved with group 0's consumption, three
        # superchunks deep (load sc+4 / transpose sc+1 / kproj+s8 sc) so each
        # stage has slack before its consumer.
        # prefetch the ENTIRE bf16 secondary (64KB/partition): no s_bf buffer
        # reuse -> load descriptor-gen is never gated on PE progress and the
        # s8cast never waits on a load at the DVE queue head (the
        # wait-late-load -> relu-late -> PE-idle -> HAM-cold-clock spiral
        # that capped the v7 loop at ~10us/superchunk)
        for _sc in range(2, N_SUPER):
            emit_load(_sc)
        # qproj h1 is emitted BETWEEN the phase-0 transposes: its pt/pq
        # matmuls give the serial T -> drain -> T -> kproj proj-bank chain
        # PE work to hide each drain behind
        emit_qproj(0, interleave=True)
        # phase-0 transposes alternate proj/gram banks (gram is free until
        # group 0's accumulator, allocated below AFTER the gram-tagged Ts
        # so the rotation order stays acyclic), and the first pair's score
        # matmuls sit between kproj and T(1,0) to window the kt drain
        emit_T(0, 0)
        emit_T(0, 1, t_tag="gram")
        emit_kproj(0)
        sc0 = emit_scores_pair(0, 0)
        emit_T(1, 0)
        emit_T(1, 1, t_tag="gram")
        emit_T(2, 0)
        st0 = {
            "ctx_ps": [psum.tile([P, E], F32, tag=f"ctx{jb}", name=f"ctx{jb}")
                       for jb in range(NB)],
            "gram_ps": psum.tile([P, NB * P], F32, tag="gram", name="gram_ps"),
            "sc": sc0,
        }
        # Steady state per iteration: kproj(sc+1), pair, T(sc+2,jp1), pair,
        # T(sc+3,jp0) - each proj-bank tenant's drain hides behind a full
        # pair of matmuls before the next tenant writes the bank.  kproj
        # must be EMITTED before the second pair: that pair pipelines the
        # next superchunk's scores matmuls, and Tile dependencies follow
        # emission order - a read emitted before its writer sees stale data.
        for sc in range(N_SUPER):
            if sc + 1 < N_SUPER:
                emit_kproj(sc + 1)
            emit_pair(0, st0, sc * PPS)
            if sc + 2 < N_SUPER:
                emit_T(sc + 2, 1)
            if sc == N_SUPER - 2:
                # qproj h1 here (group 1 is its only consumer): the last
                # three pairs' matmuls hide its transpose-drain chain, and
                # it is off the startup critical path ("proj" tag - "gram"
                # still holds group 0's accumulator)
                emit_qproj(1, ps_tags=("proj",))
            emit_pair(0, st0, sc * PPS + 1)
            if sc + 3 < N_SUPER:
                emit_T(sc + 3, 0)
        # group 1's scores start while group 0's finalize chain drains the
        # ctx/gram banks
        st1 = emit_group_prologue(1)
        emit_group_finalize(0, st0)

        for mp in range(M_PAIRS):
            emit_pair(1, st1, mp)
        emit_group_finalize(1, st1, split_copies=True)

    return nc


_NC_CACHE = None


def _get_nc():
    global _NC_CACHE
    if _NC_CACHE is None:
        nc = bacc.Bacc("TRN2", target_bir_lowering=False, debug=False)
        _emit(nc)
        nc.finalize()
        _NC_CACHE = nc
    return _NC_CACHE


def run_sharded(inputs, **kw):
    nc = _get_nc()
    prim = np.ascontiguousarray(np.asarray(inputs["primary"], dtype=np.float32))
    sec = np.ascontiguousarray(np.asarray(inputs["secondary"], dtype=np.float32))
    w = np.ascontiguousarray(np.asarray(inputs["W"], dtype=np.float32))
    b = np.ascontiguousarray(np.asarray(inputs["b"], dtype=np.float32))
    assert prim.shape == (N, E) and sec.shape == (M, E)
    assert w.shape == (L, E) and b.shape == (L,)
    in_maps = [
        {
            "primary": prim[i * NLOC:(i + 1) * NLOC],
            "secondary": sec,
            "W": w,
            "b": b,
        }
        for i in range(N_CORES)
    ]
    res = run_bass_kernel_spmd(nc, in_maps, list(range(N_CORES)), **kw)
    out = np.concatenate([res.results[i]["out"] for i in range(N_CORES)], axis=0)
    return out, res


def kernel(**inputs) -> np.ndarray:
    out, _ = run_sharded(inputs)
    return out


# revision 44
# speedup vs baseline: 1.0643x; 1.0643x over previous
"""Trainium2 Bass kernel for nn_Attention_56169582297517.

ref:  q = primary @ W.T + b            [N,L]
      k = secondary @ W.T + b          [M,L]
      s = relu(q @ k.T)                [N,M]
      s = s / max(||s||_row, 1e-12)
      out = s @ secondary              [N,E]

N=M=8192, E=512, L=128.  Sharding: primary rows split across 8 cores
(1024 rows each); secondary/W/b replicated; each core computes its row
slice independently (row-wise L2 norm is local to N).

Per-core plan (normalization deferred to the very end):
  out_row = (relu(q k^T) @ S)_row / max(norm_row, eps)

Scores are computed TRANSPOSED (m on partitions, n on free) so the
context matmul can contract m on partitions against natural-layout
secondary chunks.

The context matmul runs in fp8e4 with perf_mode=DoubleRow: two m-chunks
packed per matmul ([K=128, 2, *] APs on both operands), doubling
contraction throughput.  Scores are relu'd + cast to fp8 by the DVE
(ACT cannot write fp8 - hangs the exec unit); max score ~112 < 240 so
no scaling is needed.

The row norm is ALSO computed on the PE: a second DoubleRow matmul per
(pair, n-block) accumulates the gram diag blocks st8^T @ st8 into one
PSUM bank; the diagonal (= sum_m s^2 over the same fp8 values the ctx
matmul uses) is extracted at group end by scalar_tensor_tensor against
the identity with accum_out.

vs the 234us baseline (now ~202us):
- All inputs stream in as bf16 via SWDGE cast-DMAs (gpsimd ring): the
  f32->bf16 casts ride the DMA CME units, removing the per-superchunk
  ACT bf16-cast (1.9us) and halving the SBUF-side DMA write bytes.
- s8 (the fp8 secondary for the ctx matmul) loads DIRECTLY as a second
  SWDGE f32->fp8 cast-DMA.  A DVE cast op would sit in the strict-FIFO
  DVE queue ahead of later relus; the Tile scheduler (whose cost model
  underestimates SWDGE completion latency) hoists those far ahead, and
  on HW each late load then head-of-line blocks the relu stream and
  stalls the PE 2-4us (and the idles re-throttle the PE clock 2.4 ->
  1.2GHz, compounding).  The extra 16MB HBM read rides free bandwidth.
- The ENTIRE bf16 secondary is prefetched into SBUF (64KB/partition,
  no s_bf buffer reuse): load descriptor-gen is never gated on PE
  progress (the reuse dependency chained Q7 gen to the PE semaphore and
  collapsed the pipeline to a 10us/superchunk lockstep).
- All transposes on the PE.  XBAR dma-transpose is unusable here: each
  InstDmaTranspose acts as a barrier in the scheduled DMA stream (waits
  for all prior DMAs, blocks all later ones), serializing against the
  continuous load traffic, and concurrent XBAR transposes on both HWDGE
  rings race and corrupt data.
- gram matmuls are NOT DoubleRow: a DR LDWEIGHTS loads 256 columns
  (~213ns) against a ~53ns FD=128 matmul; with the 4 ctx DR loads per
  pair already in flight this made pairs LDWEIGHTS-pipe-bound (2.06us
  vs 1.82 matmul-stream).  Two plain fp8 accumulating matmuls per
  block get FWL (~27ns) instead.
- The proj PSUM bank rotates kproj(sc+1) -> T(sc+2,jp1) -> T(sc+3,jp0)
  across one loop iteration so every tenant's ACT drain hides behind a
  full pair of matmuls (the v1 phase-lock stalled the PE 780ns/SC).
- b loads as one contiguous [1,128] descriptor and is transposed to
  [128,1] by a K=1 PE matmul (was: 128 4-byte DMA descriptors).
- Output stores batched: one [128, NB, E] tile per group -> 8KB
  contiguous per partition, split in two halves so the first store
  overlaps the second half's scaling.
- qproj h1 runs inside the last group-0 iteration (group 1 is its only
  consumer), off the startup critical path.
- Group boundary: the next group's prologue (scores) is emitted before
  the previous group's finalize, and the last pair's gram matmuls are
  emitted before its ctx matmuls, so the norm/scale/store chain
  overlaps the next group's score/relu pipeline.

PSUM budget (8 banks): proj(1) + scores(2) + ctx(4) + gram(1) = 8.
"""

import sys
import types

import numpy as np
from contextlib import ExitStack

import concourse.bass as bass
import concourse.bacc as bacc
import concourse.mybir as mybir
import concourse.tile as tile
from concourse.bass_utils import run_bass_kernel_spmd
from concourse.masks import make_identity


def _install_ntff_shim():
    """Some images lack antenv.axon_hooks; synthesize it so
    run_bass_kernel_spmd(trace=True) (or BASS_TRACE=1) can't crash on the
    import, and wire the NTFF profile hook when the axon .so supports it."""
    if "antenv.axon_hooks" in sys.modules:
        return
    try:
        import antenv
        import antenv.axon_hooks  # noqa: F401
        return  # real module exists
    except ImportError:
        pass
    try:
        mod = types.ModuleType("antenv.axon_hooks")
        mod._hook = None
        mod.set_axon_ntff_profile_hook = lambda h: setattr(mod, "_hook", h)
        mod.get_axon_ntff_profile_hook = lambda: mod._hook
        sys.modules["antenv.axon_hooks"] = mod
        antenv.axon_hooks = mod
        try:
            from trn_agent_boot.trn_boot import _ntff_profile_via_ctypes

            hook = _ntff_profile_via_ctypes("/opt/axon/libaxon_pjrt.so")
            if hook is not None:
                mod.set_axon_ntff_profile_hook(hook)
        except Exception:
            pass
    except Exception:
        pass


_install_ntff_shim()

N_CORES = 8
N, M, E, L = 8192, 8192, 512, 128
NLOC = N // N_CORES          # 1024 primary rows per core
P = 128
EC = E // P                  # 4 e-chunks of 128
M_CHUNKS = M // P            # 64 m-chunks of 128
M_PAIRS = M_CHUNKS // 2      # 32 fp8 DoubleRow pairs
SC = 4                       # m-chunks per load superchunk (512 rows)
N_SUPER = M_CHUNKS // SC     # 16
PPS = SC // 2                # pairs per superchunk (2)
NG = 512                     # n-group width (psum free dim)
N_GROUPS = NLOC // NG        # 2
NB = NG // P                 # 4 n-blocks of 128 per group
EPS = 1e-12

F32 = mybir.dt.float32
BF16 = mybir.dt.bfloat16
FP8 = mybir.dt.float8e4
AF = mybir.ActivationFunctionType
ALU = mybir.AluOpType
DR = mybir.MatmulPerfMode.DoubleRow


def _emit(nc: bass.Bass):
    prim = nc.dram_tensor("primary", [NLOC, E], F32, kind="ExternalInput")
    sec = nc.dram_tensor("secondary", [M, E], F32, kind="ExternalInput")
    w_d = nc.dram_tensor("W", [L, E], F32, kind="ExternalInput")
    b_d = nc.dram_tensor("b", [L], F32, kind="ExternalInput")
    out_d = nc.dram_tensor("out", [NLOC, E], F32, kind="ExternalOutput")

    with tile.TileContext(nc) as tc, ExitStack() as ctx:
        consts = ctx.enter_context(tc.tile_pool(name="consts", bufs=1))
        big = ctx.enter_context(tc.tile_pool(name="big", bufs=1))
        stage = ctx.enter_context(tc.tile_pool(name="stage", bufs=2))
        work = ctx.enter_context(tc.tile_pool(name="work", bufs=3))
        psum = ctx.enter_context(tc.tile_pool(name="psum", bufs=1, space="PSUM"))

        # ---------------- constants ----------------
        ident = consts.tile([P, P], F32)
        make_identity(nc, ident)
        ident_bf = consts.tile([P, P], BF16)
        make_identity(nc, ident_bf)

        # W: one SWDGE cast-load (f32->bf16) + PE transposes.  NO XBAR
        # dma-transposes anywhere: each one acts as a barrier in the
        # scheduled DMA stream (it waits for all previously scheduled DMAs
        # and blocks all later ones), which serializes against the
        # continuous SWDGE load traffic.
        w_bf = consts.tile([P, E], BF16)
        nc.gpsimd.dma_start(w_bf, w_d[:])
        wt = consts.tile([P, EC, P], BF16)       # wt[e', ec, l] = W[l, ec*128+e']
        for e in range(EC):
            tp = psum.tile([P, P], BF16, tag="gram", name="tp")
            nc.tensor.transpose(tp, w_bf[:, e * P:(e + 1) * P], ident_bf)
            nc.scalar.copy(wt[:, e, :], tp)

        # b: contiguous [1,128] load, then a K=1 matmul against ident[0,0]=1
        # puts b on partitions ([128,1]) for the activation bias reads
        b_row = consts.tile([1, L], F32)
        nc.sync.dma_start(b_row, b_d[:].rearrange("(o l) -> o l", o=1))
        b_ps = psum.tile([P, 1], F32, tag="gram", name="b_ps")
        nc.tensor.matmul(b_ps, lhsT=b_row, rhs=ident[0:1, 0:1], start=True, stop=True)
        b_sb = consts.tile([P, 1], F32)
        nc.scalar.copy(b_sb, b_ps)

        # ------------- secondary stream state -------------
        s8 = big.tile([P, M_PAIRS, 2, E], FP8)     # [m_in, pair, j, e]
        kt = big.tile([P, M], BF16)                # [l, m]
        s_bfs = {}
        st_sbs = {}

        def emit_load(sc):
            # SWDGE cast-DMA: f32 HBM -> bf16 SBUF.  Partition p holds DRAM
            # rows 4p+j (j inner): 8KB contiguous reads per partition.  One
            # 1MB dma per superchunk: SWDGE descriptor-gen is paced by lane
            # reuse (8 lanes x ~2us completion), so fewer/bigger ops drain
            # the upfront prefetch queue faster.  The m<->partition
            # permutation is absorbed by construction: kt columns, st8
            # partitions and s8 partitions all inherit it from this same
            # load, and m is fully contracted.
            s_bf = stage.tile([P, SC, E], BF16, tag="sbf", name="s_bf", bufs=16)
            base = sec[sc * SC * P:(sc + 1) * SC * P, :].rearrange("(p j) e -> p j e", j=SC)
            nc.gpsimd.dma_start(s_bf, base)
            s_bfs[sc] = s_bf
            emit_s8load(sc)

        def emit_T(sc, jp, t_tag="proj"):
            # PE transposes of two m-chunks.  The proj PSUM bank rotates
            # kproj(sc+1) -> T(sc+2,jp1) -> T(sc+3,jp0) across one loop
            # iteration, so every tenant's drain has a full pair of score/
            # ctx matmuls in front of the next tenant's first write - the
            # v1 phase-lock (T, drain, T, drain, kproj back-to-back) stalled
            # the PE 780ns+ per superchunk waiting on the ACT drains.
            s_bf = s_bfs[sc]
            st_ps = psum.tile([P, EC, 2 * P], BF16, tag=t_tag, name="st_ps")
            for jj in range(2):
                j = jp * 2 + jj
                for e in range(EC):
                    nc.tensor.transpose(
                        st_ps[:, e, jj * P:(jj + 1) * P],
                        s_bf[:, j, e * P:(e + 1) * P],
                        ident_bf,
                    )
            if sc not in st_sbs:
                st_sbs[sc] = stage.tile([P, EC, SC * P], BF16, tag="st", name="st_sb", bufs=3)
            dst = st_sbs[sc][:, :, jp * 2 * P:(jp + 1) * 2 * P]
            # both drains on ACT: the DVE queue must stay pure relu+s8cast,
            # a drain queued behind the relus would stall kproj on the PE
            nc.scalar.copy(dst, st_ps)

        def emit_kproj(sc):
            st_sb = st_sbs.pop(sc)
            pk = psum.tile([P, SC * P], F32, tag="proj", name="pk")
            for e in range(EC):
                nc.tensor.matmul(
                    pk,
                    lhsT=wt[:, e, :],
                    rhs=st_sb[:, e, :],
                    start=(e == 0),
                    stop=(e == EC - 1),
                )
            nc.scalar.activation(kt[:, sc * SC * P:(sc + 1) * SC * P], pk, AF.Identity, bias=b_sb)

        def emit_s8load(sc):
            # s8 loads DIRECTLY as a second SWDGE cast-DMA (f32 HBM -> fp8
            # SBUF).  A DVE bf16->fp8 cast op would sit in the strict-FIFO
            # DVE queue ahead of later relus; the Tile scheduler (whose cost
            # model underestimates SWDGE completion latency) hoists these
            # far ahead, and on HW each late load then head-of-line blocks
            # the relu stream and stalls the PE 2-4us.  The extra 16MB HBM
            # read rides free bandwidth.
            base = sec[sc * SC * P:(sc + 1) * SC * P, :].rearrange("(p j) e -> p j e", j=SC)
            nc.gpsimd.dma_start(s8[:, sc * PPS:(sc + 1) * PPS, :, :], base)

        # ---------------- qT = W @ P_loc^T + b  -> [l, n]  (bf16) ----------------
        # SWDGE cast-load (partition p holds prim rows h*512+4p+j -> qt
        # column j*128+p), PE transposes (gram bank is idle until the main
        # loop), then the e-contraction on PE.
        qt = big.tile([P, NLOC], BF16)
        pc_bfs = []
        for h in range(NLOC // NG):
            # Pool-ring order pc0, L0, pc1, L1: each consumer's load lands
            # just ahead of its first use in the startup chain
            pc_bf = stage.tile([P, NB, E], BF16, tag="pchunk", name="pc_bf")
            nc.gpsimd.dma_start(
                pc_bf, prim[h * NG:(h + 1) * NG, :].rearrange("(p j) e -> p j e", j=NB))
            pc_bfs.append(pc_bf)
            emit_load(h)

        def emit_qproj(h, ps_tags=("gram", "proj"), interleave=False):
            # alternate PSUM banks for the transpose blocks so consecutive
            # drains overlap (a single bank serializes T -> drain -> T ...).
            # interleave=True (startup h0 only): pq lives in a scores bank
            # (free at startup) and each block's projection matmuls are
            # emitted right after the previous block's drain, filling every
            # bank-handoff window with PE work.  Not for h1: the scores
            # banks are hot mid-loop, and pq-at-top in the proj rotation
            # would deadlock against the later pt blocks.
            pc_bf = pc_bfs[h]
            pt_sb = stage.tile([P, NB, EC, P], BF16, tag="pt", name="pt_sb")
            pq = psum.tile([P, NG], F32, tag="scores" if interleave else "proj",
                           name="pq", bufs=2 if interleave else 1)

            def pq_block(nb4):
                for e in range(EC):
                    nc.tensor.matmul(
                        pq[:, nb4 * P:(nb4 + 1) * P],
                        lhsT=wt[:, e, :],
                        rhs=pt_sb[:, nb4, e, :],
                        start=(e == 0),
                        stop=(e == EC - 1),
                    )

            for nb4 in range(NB):
                pt_ps = psum.tile([P, EC, P], BF16, tag=ps_tags[nb4 % len(ps_tags)], name="pt_ps")
                for e in range(EC):
                    nc.tensor.transpose(
                        pt_ps[:, e, :], pc_bf[:, nb4, e * P:(e + 1) * P], ident_bf)
                if nb4 % 2 == 0:
                    nc.scalar.copy(pt_sb[:, nb4, :, :], pt_ps)
                else:
                    nc.vector.tensor_copy(pt_sb[:, nb4, :, :], pt_ps)
                if interleave and nb4 >= 1:
                    pq_block(nb4 - 1)
            if interleave:
                pq_block(NB - 1)
            else:
                for nb4 in range(NB):
                    pq_block(nb4)
            nc.scalar.activation(qt[:, h * NG:(h + 1) * NG], pq, AF.Identity, bias=b_sb)

        # ---------------- main loop: scores^T, gram norms, context ----------------
        def emit_scores_pair(g, mp):
            tiles = []
            for j in range(2):
                sc_ps = psum.tile([P, NG], F32, tag="scores", name="sc_ps", bufs=2)
                nc.tensor.matmul(
                    sc_ps,
                    lhsT=kt[:, (2 * mp + j) * P:(2 * mp + j + 1) * P],
                    rhs=qt[:, g * NG:(g + 1) * NG],
                    start=True,
                    stop=True,
                )
                tiles.append(sc_ps)
            return tiles

        def emit_group_prologue(g):
            ctx_ps = [
                psum.tile([P, E], F32, tag=f"ctx{jb}", name=f"ctx{jb}") for jb in range(NB)
            ]
            gram_ps = psum.tile([P, NB * P], F32, tag="gram", name="gram_ps")
            return {"ctx_ps": ctx_ps, "gram_ps": gram_ps,
                    "sc": emit_scores_pair(g, 0)}

        def emit_pair(g, st, mp, split_relu=False):
            st8 = work.tile([P, 2, NG], FP8, tag="st8", name="st8", bufs=4)
            # relu + fp8 cast on DVE (ACT cannot write fp8).  In group 0 the
            # relu is split into column halves: the first ctx/gram matmuls
            # read only st8[:, :, 0:256], so the PE's per-superchunk relu
            # edge shrinks by ~half a relu.  Group 1 keeps whole-row relus
            # (its pair cadence is tighter and the extra DVE op overhead
            # would make the DVE the bottleneck).
            H = NG // 2
            if split_relu:
                nc.vector.tensor_scalar_max(st8[:, 0, 0:H], st["sc"][0][:, 0:H], 0.0)
                nc.vector.tensor_scalar_max(st8[:, 1, 0:H], st["sc"][1][:, 0:H], 0.0)
                nc.vector.tensor_scalar_max(st8[:, 0, H:NG], st["sc"][0][:, H:NG], 0.0)
                nc.vector.tensor_scalar_max(st8[:, 1, H:NG], st["sc"][1][:, H:NG], 0.0)
            else:
                nc.vector.tensor_scalar_max(st8[:, 0, :], st["sc"][0], 0.0)
                nc.vector.tensor_scalar_max(st8[:, 1, :], st["sc"][1], 0.0)
            # next pair's scores issued ahead of the ctx matmuls in the
            # in-order PE stream (they run as soon as the relus drain the
            # score banks, feeding the next relus)
            if mp + 1 < M_PAIRS:
                st["sc"] = emit_scores_pair(g, mp + 1)
            last = mp == M_PAIRS - 1

            def emit_ctx(jb, lhsT):
                nc.tensor.matmul(
                    st["ctx_ps"][jb],
                    lhsT=lhsT,
                    rhs=s8[:, mp, :, :],
                    start=(mp == 0),
                    stop=last,
                    perf_mode=DR,
                )

            def emit_gram(jb, lhsT):
                # row-norm accumulation: gram diag block, same values as the
                # ctx lhsT.  Mixed perf modes: a DR LDWEIGHTS loads 256
                # columns (~213ns) vs a ~53ns matmul at FD=128, so all-DR
                # made pairs LDWEIGHTS-pipe-bound (2.06us vs 1.82 matmul
                # stream) while all-plain doubles the gram matmul cycles.
                # Two DR blocks + two plain-FWL blocks lands both streams
                # under the pipe limits (LDW 1.49us, MM 1.66us per pair).
                # The PSUM zero region is the whole 2KB bank, so only the
                # FIRST gram matmul may carry start=True: a start on a later
                # one would clear has_written for the already-written
                # regions and their next write would overwrite, silently
                # dropping pair 0 from those rows' norms.
                if jb % 2 == 0:
                    nc.tensor.matmul(
                        st["gram_ps"][:, jb * P:(jb + 1) * P],
                        lhsT=lhsT,
                        rhs=lhsT,
                        start=(mp == 0 and jb == 0),
                        stop=False,
                        perf_mode=DR,
                        skip_group_check=True,
                    )
                else:
                    for j in range(2):
                        nc.tensor.matmul(
                            st["gram_ps"][:, jb * P:(jb + 1) * P],
                            lhsT=lhsT[:, j, :],
                            rhs=lhsT[:, j, :],
                            start=False,
                            stop=(last and jb == NB - 1 and j == 1),
                            skip_group_check=True,
                        )

            for jb in range(NB):
                lhsT = st8[:, :, jb * P:(jb + 1) * P]
                if last:
                    # gram first on the last pair: the gram bank completes
                    # ~0.9us earlier, so the finalize norm chain overlaps
                    # the remaining ctx matmuls
                    emit_gram(jb, lhsT)
                else:
                    emit_ctx(jb, lhsT)
                    emit_gram(jb, lhsT)
            if last:
                for jb in range(NB):
                    emit_ctx(jb, st8[:, :, jb * P:(jb + 1) * P])

        def emit_group_finalize(g, st, split_copies=False):
            # ------- out = ctx / max(sqrt(diag(gram)), eps) -------
            o_raw = None
            if not split_copies:
                # raw ctx drains first: free the banks for the next group.
                # Split ACT/DVE - these are gated only on the ctx matmuls
                # (not the norm chain), so the DVE ops cannot head-of-line
                # block the next group's relus
                o_raw = work.tile([P, NB, E], F32, tag="oraw", name="o_raw", bufs=1)
                nc.scalar.copy(o_raw[:, 0, :], st["ctx_ps"][0])
                nc.vector.tensor_copy(o_raw[:, 2, :], st["ctx_ps"][2])
                nc.scalar.copy(o_raw[:, 1, :], st["ctx_ps"][1])
                nc.vector.tensor_copy(o_raw[:, 3, :], st["ctx_ps"][3])
            n2 = work.tile([P, NB], F32, tag="n2", name="n2", bufs=1)
            for jb in range(NB):
                scratch = work.tile([P, P], F32, tag="scr", name="scratch", bufs=2)
                nc.vector.scalar_tensor_tensor(
                    scratch, st["gram_ps"][:, jb * P:(jb + 1) * P], 1.0, ident,
                    ALU.mult, ALU.mult, accum_out=n2[:, jb:jb + 1],
                )
            nrm = work.tile([P, NB], F32, tag="nrm", name="nrm", bufs=1)
            nc.scalar.activation(nrm, n2, AF.Sqrt)
            nrm_c = work.tile([P, NB], F32, tag="nrmc", name="nrm_c", bufs=1)
            nc.vector.tensor_scalar_max(nrm_c, nrm, EPS)
            recip = work.tile([P, NB], F32, tag="recip", name="recip", bufs=1)
            nc.vector.reciprocal(recip, nrm_c)
            # batched store: o_sb partition p holds rows g*512 + 4p + jb (the
            # primary-load permutation) -> 8KB contiguous per partition,
            # split in two halves so store 0 overlaps the jb=2,3 scaling
            o_sb = work.tile([P, NB, E], F32, tag="osb", name="o_sb", bufs=2)
            out_blk = out_d[g * NG:(g + 1) * NG, :].rearrange("(p j) e -> p j e", j=NB)
            if split_copies:
                # final group only: scale-copies split ACT (jb 0,1) / DVE
                # (jb 2,3) so the two halves run in parallel - the serial
                # 2.9us ACT chain was the fattest piece of the exit tail.
                # NOT for the mid-kernel finalize: a DVE op gated on the
                # norm chain would head-of-line block the next group's
                # relus in the strict-FIFO DVE queue.
                nc.scalar.activation(o_sb[:, 0, :], st["ctx_ps"][0], AF.Copy,
                                     scale=recip[:, 0:1])
                nc.vector.tensor_scalar_mul(o_sb[:, 2, :], st["ctx_ps"][2], recip[:, 2:3])
                nc.scalar.activation(o_sb[:, 1, :], st["ctx_ps"][1], AF.Copy,
                                     scale=recip[:, 1:2])
                nc.vector.tensor_scalar_mul(o_sb[:, 3, :], st["ctx_ps"][3], recip[:, 3:4])
                nc.scalar.dma_start(out_blk[:, 0:2, :], o_sb[:, 0:2, :])
                nc.scalar.dma_start(out_blk[:, 2:4, :], o_sb[:, 2:4, :])
            else:
                # mid-kernel finalize: drain the ctx banks RAW first (gated
                # only on the last ctx matmul, not on the norm chain) so the
                # next group's start=True ctx matmuls get their banks ~3us
                # earlier; the scale then applies SBUF->SBUF off-path
                for jb in range(NB):
                    nc.scalar.activation(o_sb[:, jb, :], o_raw[:, jb, :], AF.Copy,
                                         scale=recip[:, jb:jb + 1])
                    if jb == 1:
                        nc.scalar.dma_start(out_blk[:, 0:2, :], o_sb[:, 0:2, :])
                nc.scalar.dma_start(out_blk[:, 2:4, :], o_sb[:, 2:4, :])

        # Phase-0 production interleaved with group 0's consumption, three
        # superchunks deep (load sc+4 / transpose sc+1 / kproj+s8 sc) so each
        # stage has slack before its consumer.
        # prefetch the ENTIRE bf16 secondary (64KB/partition): no s_bf buffer
        # reuse -> load descriptor-gen is never gated on PE progress and the
        # s8cast never waits on a load at the DVE queue head (the
        # wait-late-load -> relu-late -> PE-idle -> HAM-cold-clock spiral
        # that capped the v7 loop at ~10us/superchunk)
        for _sc in range(2, N_SUPER):
            emit_load(_sc)
        # qproj h1 is emitted BETWEEN the phase-0 transposes: its pt/pq
        # matmuls give the serial T -> drain -> T -> kproj proj-bank chain
        # PE work to hide each drain behind
        emit_qproj(0, interleave=True)
        # phase-0 transposes alternate proj/gram banks (gram is free until
        # group 0's accumulator, allocated below AFTER the gram-tagged Ts
        # so the rotation order stays acyclic), and the first pair's score
        # matmuls sit between kproj and T(1,0) to window the kt drain
        emit_T(0, 0)
        emit_T(0, 1, t_tag="gram")
        emit_kproj(0)
        sc0 = emit_scores_pair(0, 0)
        emit_T(1, 0)
        emit_T(1, 1, t_tag="gram")
        emit_T(2, 0)
        st0 = {
            "ctx_ps": [psum.tile([P, E], F32, tag=f"ctx{jb}", name=f"ctx{jb}")
                       for jb in range(NB)],
            "gram_ps": psum.tile([P, NB * P], F32, tag="gram", name="gram_ps"),
            "sc": sc0,
        }
        # Steady state per iteration: kproj(sc+1), pair, T(sc+2,jp1), pair,
        # T(sc+3,jp0) - each proj-bank tenant's drain hides behind a full
        # pair of matmuls before the next tenant writes the bank.  kproj
        # must be EMITTED before the second pair: that pair pipelines the
        # next superchunk's scores matmuls, and Tile dependencies follow
        # emission order - a read emitted before its writer sees stale data.
        for sc in range(N_SUPER):
            if sc + 1 < N_SUPER:
                emit_kproj(sc + 1)
            emit_pair(0, st0, sc * PPS)
            if sc + 2 < N_SUPER:
                emit_T(sc + 2, 1)
            if sc == N_SUPER - 2:
                # qproj h1 here (group 1 is its only consumer): the last
                # three pairs' matmuls hide its transpose-drain chain, and
                # it is off the startup critical path ("proj" tag - "gram"
                # still holds group 0's accumulator)
                emit_qproj(1, ps_tags=("proj",))
            emit_pair(0, st0, sc * PPS + 1)
            if sc + 3 < N_SUPER:
                emit_T(sc + 3, 0)
        # group 1's scores start while group 0's finalize chain drains the
        # ctx/gram banks
        st1 = emit_group_prologue(1)
        emit_group_finalize(0, st0)

        for mp in range(M_PAIRS):
            emit_pair(1, st1, mp)
        emit_group_finalize(1, st1, split_copies=True)

    return nc


_NC_CACHE = None


def _get_nc():
    global _NC_CACHE
    if _NC_CACHE is None:
        nc = bacc.Bacc("TRN2", target_bir_lowering=False, debug=False)
        _emit(nc)
        nc.finalize()
        _NC_CACHE = nc
    return _NC_CACHE


def run_sharded(inputs, **kw):
    nc = _get_nc()
    prim = np.ascontiguousarray(np.asarray(inputs["primary"], dtype=np.float32))
    sec = np.ascontiguousarray(np.asarray(inputs["secondary"], dtype=np.float32))
    w = np.ascontiguousarray(np.asarray(inputs["W"], dtype=np.float32))
    b = np.ascontiguousarray(np.asarray(inputs["b"], dtype=np.float32))
    assert prim.shape == (N, E) and sec.shape == (M, E)
    assert w.shape == (L, E) and b.shape == (L,)
    in_maps = [
        {
            "primary": prim[i * NLOC:(i + 1) * NLOC],
            "secondary": sec,
            "W": w,
            "b": b,
        }
        for i in range(N_CORES)
    ]
    res = run_bass_kernel_spmd(nc, in_maps, list(range(N_CORES)), **kw)
    out = np.concatenate([res.results[i]["out"] for i in range(N_CORES)], axis=0)
    return out, res


def kernel(**inputs) -> np.ndarray:
    out, _ = run_sharded(inputs)
    return out
